# revision 5
# baseline (speedup 1.0000x reference)
"""Self-contained Trainium2 Bass kernel: 16-head causal attention with RoPE.

Sharding: tensor-parallel over heads (16 heads / 8 cores = 2 heads per core).
Each core computes the qkv projection for its 2 heads, causal flash
attention, and a partial output projection (w_o columns for its heads); the
8 partial [B*L, H] outputs are summed on the host.

Layout strategy (avoids all on-device transposes):
  - x is passed host-transposed as xT [H, B*L].
  - q, k are computed head-dim-major  qT/kT [128, L]  (d on partitions).
  - v is computed token-major v [L, 128] (tokens on partitions).
  - RoPE: rope(q) = q * cosT + R @ (q * sinT), with R a constant 128x128
    +-1 half-rotation matrix applied on the tensor engine (works because the
    sin table is identical for paired dims d and d+64).
  - scores are computed transposed: scT [k_tok, q_tok] = kT_chunk.T @ qT.
    No max subtraction (scores are O(+-6) for this data distribution; exact
    softmax up to fp32 rounding), exp on the scalar engine, causal masking by
    only computing the live column ranges + one triangular mask multiply per
    diagonal 128x128 block.
  - attn@v: out = (v_chunk as lhsT).T @ expT accumulated over k chunks ->
    output is head-dim-major [128, q], which is exactly the lhsT layout the
    w_o projection needs.
  - softmax denominator: ones[128,1].T @ expT accumulated in PSUM,
    reciprocal, broadcast across partitions via a K=1 matmul outer product,
    applied in the PSUM->SBUF normalize multiply.

Matmul dtype: float32r (TRN2 streams f32r at 1 row/cycle vs 4 for f32).
All matmul operands are produced natively as f32r (walrus requires rounded
producers); PSUM accumulation stays fp32.
"""

import numpy as np
from contextlib import ExitStack

import concourse.bass as bass
import concourse.tile as tile
from concourse import bacc, mybir
from concourse.bass_utils import run_bass_kernel_spmd
from concourse.masks import make_upper_triangular

F32 = mybir.dt.float32
F32R = mybir.dt.float32r
AF = mybir.ActivationFunctionType

NCORES = 8
HD = 128
ROPE_THETA = 10000.0
USE_F32R = True


def rope_tables_T(Lsz):
    """cos/sin tables transposed to [HD, L], matching the fp32 reference."""
    half = np.arange(0, HD, 2).astype(np.float32) / np.float32(HD)
    inv_freq = (np.float32(1.0) / np.power(np.float32(ROPE_THETA), half,
                                           dtype=np.float32)).astype(np.float32)
    t = np.arange(Lsz, dtype=np.float32)
    freqs = np.outer(t, inv_freq).astype(np.float32)          # [L, HD/2]
    emb = np.concatenate([freqs, freqs], axis=1)              # [L, HD]
    cosT = np.ascontiguousarray(np.cos(emb).astype(np.float32).T)  # [HD, L]
    sinT = np.ascontiguousarray(np.sin(emb).astype(np.float32).T)
    return cosT, sinT


def rot_matrix_T():
    """R with rot(x) = R @ x = concat(-x2, x1); returns R.T (matmul lhsT)."""
    R = np.zeros((HD, HD), dtype=np.float32)
    h = HD // 2
    for d in range(h):
        R[d, d + h] = -1.0
        R[d + h, d] = 1.0
    return np.ascontiguousarray(R.T)


def build_attention_nc(Bsz, Lsz, Hsz, hpc, use_f32r=USE_F32R):
    """Build + compile the per-core Bass program (identical on all cores)."""
    f = Hsz // 128            # feature chunks of the model dim
    nt = Bsz * Lsz            # total tokens
    dloc = hpc * HD           # local head dims
    RC = 512                  # token chunk for projection + rope
    XC = 256                  # token chunk for x streaming (2 halves per RC)
    QT = 512                  # q tile for attention
    KCL = Lsz // 128          # k chunks per sequence
    scale = float(1.0 / np.sqrt(HD))
    MD = F32R if use_f32r else F32   # dtype for all matmul operands

    nc = bacc.Bacc("TRN2", target_bir_lowering=False, debug=False)

    xT = nc.dram_tensor("xT", [Hsz, nt], MD, kind="ExternalInput").ap()
    wqT = nc.dram_tensor("wqT", [Hsz, dloc], MD, kind="ExternalInput").ap()
    wkT = nc.dram_tensor("wkT", [Hsz, dloc], MD, kind="ExternalInput").ap()
    wvT = nc.dram_tensor("wvT", [Hsz, dloc], MD, kind="ExternalInput").ap()
    woT = nc.dram_tensor("woT", [dloc, Hsz], MD, kind="ExternalInput").ap()
    cosT = nc.dram_tensor("cosT", [HD, Lsz], F32, kind="ExternalInput").ap()
    sinT = nc.dram_tensor("sinT", [HD, Lsz], F32, kind="ExternalInput").ap()
    RT = nc.dram_tensor("RT", [HD, HD], MD, kind="ExternalInput").ap()
    y = nc.dram_tensor("y", [nt, Hsz], F32, kind="ExternalOutput").ap()

    with tile.TileContext(nc) as tc, \
         nc.allow_low_precision(reason="f32r matmul operands"), ExitStack() as ctx:
        wpool = ctx.enter_context(tc.tile_pool(name="wpool", bufs=1))
        cpool = ctx.enter_context(tc.tile_pool(name="cpool", bufs=1))
        xpool = ctx.enter_context(tc.tile_pool(name="xpool", bufs=2))
        spool = ctx.enter_context(tc.tile_pool(name="spool", bufs=1))
        work = ctx.enter_context(tc.tile_pool(name="work", bufs=2))
        psp = ctx.enter_context(tc.tile_pool(name="psp", bufs=2, space="PSUM"))

        # --- constants / weights resident in SBUF ---
        wq_s = wpool.tile([128, f, dloc], MD)
        wk_s = wpool.tile([128, f, dloc], MD)
        wv_s = wpool.tile([128, f, dloc], MD)
        wo_s = wpool.tile([128, hpc, Hsz], MD)
        nc.sync.dma_start(out=wq_s, in_=wqT.rearrange("(c p) m -> p c m", p=128))
        nc.sync.dma_start(out=wk_s, in_=wkT.rearrange("(c p) m -> p c m", p=128))
        nc.sync.dma_start(out=wv_s, in_=wvT.rearrange("(c p) m -> p c m", p=128))
        nc.sync.dma_start(out=wo_s, in_=woT.rearrange("(h p) n -> p h n", p=128))

        cos_s = cpool.tile([128, Lsz], F32)
        sin_s = cpool.tile([128, Lsz], F32)
        rt_s = cpool.tile([128, 128], MD)
        nc.sync.dma_start(out=cos_s, in_=cosT)
        nc.sync.dma_start(out=sin_s, in_=sinT)
        nc.sync.dma_start(out=rt_s, in_=RT)
        tri_f = cpool.tile([128, 128], F32)
        make_upper_triangular(nc, tri_f, val=1.0, diag=True)
        ones_f = cpool.tile([128, 1], F32)
        nc.vector.memset(ones_f, 1.0)
        ones1_f = cpool.tile([1, 128], F32)
        nc.vector.memset(ones1_f, 1.0)
        if use_f32r:
            tri_s = cpool.tile([128, 128], MD)
            nc.vector.tensor_copy(tri_s, tri_f)
            ones_s = cpool.tile([128, 1], MD)
            nc.vector.tensor_copy(ones_s, ones_f)
            ones1_s = cpool.tile([1, 128], MD)
            nc.vector.tensor_copy(ones1_s, ones1_f)
        else:
            tri_s, ones_s, ones1_s = tri_f, ones_f, ones1_f

        for b in range(Bsz):
            tb = b * Lsz  # token offset of this batch in xT / y

            # per-batch activation tensors (tags reused across b iterations)
            q_s = spool.tile([128, hpc, Lsz], MD, tag="q_s")
            k_s = spool.tile([128, hpc, Lsz], MD, tag="k_s")
            v_s = spool.tile([128, KCL, hpc, 128], MD, tag="v_s")
            o_s = spool.tile([128, hpc, Lsz], MD, tag="o_s")

            # ---------------- P1: qkv projection + rope ----------------
            for rc in range(Lsz // RC):
                t0 = rc * RC
                xts = []
                for half in range(RC // XC):
                    xt_t = xpool.tile([128, f, XC], MD, tag="xt")
                    nc.sync.dma_start(
                        out=xt_t,
                        in_=xT.rearrange("(c p) n -> p c n", p=128)[
                            :, :, tb + t0 + half * XC: tb + t0 + (half + 1) * XC],
                    )
                    xts.append(xt_t)

                # q/k projections (head-dim-major) + rope, one (qk, h) at a time
                for dst, w_s in ((q_s, wq_s), (k_s, wk_s)):
                    for h in range(hpc):
                        p_ps = psp.tile([128, RC], F32, tag="proj", bufs=2)
                        first = True
                        for half in range(RC // XC):
                            cols = slice(half * XC, (half + 1) * XC)
                            for c in range(f):
                                nc.tensor.matmul(
                                    p_ps[:, cols],
                                    w_s[:, c, h * 128:(h + 1) * 128],
                                    xts[half][:, c, :],
                                    start=first,
                                    stop=(half == RC // XC - 1 and c == f - 1),
                                )
                                first = False
                        # rope: dst = p*cos + R @ (p*sin)
                        qs_t = work.tile([128, RC], MD, tag="ropesin", bufs=2)
                        nc.vector.tensor_mul(qs_t, p_ps, sin_s[:, t0:t0 + RC])
                        r_ps = psp.tile([128, RC], F32, tag="rot", bufs=2)
                        nc.tensor.matmul(r_ps, rt_s, qs_t, start=True, stop=True)
                        qc_t = work.tile([128, RC], F32, tag="ropecos", bufs=2)
                        nc.vector.tensor_mul(qc_t, p_ps, cos_s[:, t0:t0 + RC])
                        nc.vector.tensor_add(dst[:, h, t0:t0 + RC], qc_t, r_ps)

                # v projection (token-major, both heads at once)
                for half in range(RC // XC):
                    for m in range(XC // 128):
                        v_ps = psp.tile([128, dloc], F32, tag="vps", bufs=1)
                        for c in range(f):
                            nc.tensor.matmul(
                                v_ps,
                                xts[half][:, c, m * 128:(m + 1) * 128],
                                wv_s[:, c, :],
                                start=(c == 0), stop=(c == f - 1),
                            )
                        kc = (t0 + half * XC) // 128 + m
                        nc.scalar.activation(
                            v_s[:, kc, :, :].rearrange("p h d -> p (h d)"),
                            v_ps, AF.Copy)

            # ---------------- P2: causal attention ----------------
            for h in range(hpc):
                for qt in range(Lsz // QT):
                    q0 = qt * QT
                    nkc = (q0 + QT) // 128
                    o_ps = psp.tile([128, QT], F32, tag="oacc", bufs=1)
                    d_ps = psp.tile([1, QT], F32, tag="vps", bufs=1)
                    for kc in range(nkc):
                        diag_j = kc - q0 // 128
                        c0 = max(0, diag_j * 128)
                        sc_ps = psp.tile([128, QT], F32, tag="sc", bufs=2)
                        nc.tensor.matmul(
                            sc_ps[:, c0:],
                            k_s[:, h, kc * 128:(kc + 1) * 128],
                            q_s[:, h, q0 + c0:q0 + QT],
                            start=True, stop=True,
                        )
                        e_t = work.tile([128, QT], MD, tag="exp", bufs=3)
                        nc.scalar.activation(e_t[:, c0:], sc_ps[:, c0:],
                                             AF.Exp, scale=scale)
                        if diag_j >= 0:
                            nc.vector.tensor_mul(e_t[:, c0:c0 + 128],
                                                 e_t[:, c0:c0 + 128], tri_s)
                        nc.tensor.matmul(
                            o_ps[:, c0:],
                            v_s[:, kc, h, :],
                            e_t[:, c0:],
                            start=(kc == 0), stop=(kc == nkc - 1),
                        )
                        nc.tensor.matmul(
                            d_ps[:, c0:],
                            ones_s,
                            e_t[:, c0:],
                            start=(kc == 0), stop=(kc == nkc - 1),
                        )
                    rc_t = work.tile([1, QT], MD, tag="recip", bufs=2)
                    nc.vector.reciprocal(rc_t, d_ps)
                    rb_ps = psp.tile([128, QT], F32, tag="proj", bufs=2)
                    nc.tensor.matmul(rb_ps, ones1_s, rc_t, start=True, stop=True)
                    rb_s = work.tile([128, QT], F32, tag="rb", bufs=2)
                    nc.scalar.activation(rb_s, rb_ps, AF.Copy)
                    nc.vector.tensor_mul(o_s[:, h, q0:q0 + QT], o_ps, rb_s)

            # ---------------- P3: partial output projection ----------------
            for t in range(Lsz // 128):
                for n0 in range(0, Hsz, 512):
                    y_ps = psp.tile([128, 512], F32, tag="rot", bufs=2)
                    for h in range(hpc):
                        nc.tensor.matmul(
                            y_ps,
                            o_s[:, h, t * 128:(t + 1) * 128],
                            wo_s[:, h, n0:n0 + 512],
                            start=(h == 0), stop=(h == hpc - 1),
                        )
                    y_t = work.tile([128, 512], F32, tag="yst", bufs=3)
                    if (t + n0 // 512) % 2 == 0:
                        nc.scalar.activation(y_t, y_ps, AF.Copy)
                    else:
                        nc.vector.tensor_copy(y_t, y_ps)
                    nc.sync.dma_start(
                        out=y[tb + t * 128: tb + (t + 1) * 128, n0:n0 + 512],
                        in_=y_t)

    nc.compile()
    return nc


# ------------------------- host-side entry point -------------------------

_NC_CACHE = {}


def _get_nc(Bsz, Lsz, Hsz, hpc, use_f32r):
    key = (Bsz, Lsz, Hsz, hpc, use_f32r)
    if key not in _NC_CACHE:
        _NC_CACHE[key] = build_attention_nc(Bsz, Lsz, Hsz, hpc, use_f32r)
    return _NC_CACHE[key]


def make_in_maps(x, w_qkv, w_o, hpc):
    """Host-side sharding: per-core input dicts (all arrays np.float32)."""
    Bsz, Lsz, Hsz = x.shape
    dloc = hpc * HD
    xTh = np.ascontiguousarray(
        x.reshape(Bsz * Lsz, Hsz).T.astype(np.float32, copy=False))
    w_q, w_k, w_v = (w_qkv[i * Hsz:(i + 1) * Hsz] for i in range(3))
    cosTh, sinTh = rope_tables_T(Lsz)
    rth = rot_matrix_T()
    in_maps = []
    for c in range(NCORES):
        sl = slice(c * dloc, (c + 1) * dloc)
        in_maps.append({
            "xT": xTh,
            "wqT": np.ascontiguousarray(w_q[sl].T),
            "wkT": np.ascontiguousarray(w_k[sl].T),
            "wvT": np.ascontiguousarray(w_v[sl].T),
            "woT": np.ascontiguousarray(w_o[:, sl].T),
            "cosT": cosTh,
            "sinT": sinTh,
            "RT": rth,
        })
    return in_maps


def run(x, w_qkv, w_o, trace=False, use_f32r=USE_F32R):
    Bsz, Lsz, Hsz = x.shape
    hpc = (Hsz // HD) // NCORES
    nc = _get_nc(Bsz, Lsz, Hsz, hpc, use_f32r)
    in_maps = make_in_maps(np.asarray(x), np.asarray(w_qkv), np.asarray(w_o), hpc)
    res = run_bass_kernel_spmd(nc, in_maps, core_ids=list(range(NCORES)),
                               trace=trace)
    parts = np.stack([res.results[c]["y"] for c in range(NCORES)])
    y = parts.sum(axis=0, dtype=np.float64).astype(np.float32)
    return y.reshape(Bsz, Lsz, Hsz), res


def kernel(x, w_qkv, w_o):
    y, _ = run(x, w_qkv, w_o, trace=False)
    return y


# revision 18
# speedup vs baseline: 171.9616x; 171.9616x over previous
"""Self-contained Trainium2 Bass kernel: 16-head causal attention with RoPE.

Sharding: tensor-parallel over heads (16 heads / 8 cores = 2 heads per core).
Each core computes the qkv projection for its 2 heads, causal flash
attention, and a partial output projection (w_o columns for its heads); the
8 partial [B*L, H] outputs are summed on the host.

Layout strategy (avoids all on-device transposes):
  - x is passed host-transposed as xT [H, B*L].
  - q, k are computed head-dim-major  qT/kT [128, L]  (d on partitions).
  - v is computed token-major v [L, 128] (tokens on partitions).
  - RoPE: rope(q) = q * cosT + R @ (q * sinT), with R a constant 128x128
    +-1 half-rotation matrix applied on the tensor engine (works because the
    sin table is identical for paired dims d and d+64).
  - scores are computed transposed: scT [k_tok, q_tok] = kT_chunk.T @ qT.
    No max subtraction (scores are O(+-6) for this data distribution; exact
    softmax up to fp32 rounding), exp on the scalar engine, causal masking by
    only computing the live column ranges + one triangular mask multiply per
    diagonal 128x128 block.
  - attn@v: out = (v_chunk as lhsT).T @ expT accumulated over k chunks ->
    output is head-dim-major [128, q], which is exactly the lhsT layout the
    w_o projection needs.
  - softmax denominator: ones[128,1].T @ expT accumulated in PSUM,
    reciprocal, broadcast across partitions via a K=1 matmul outer product,
    applied in the PSUM->SBUF normalize multiply.

Matmul dtype: float32r (TRN2 streams f32r at 1 row/cycle vs 4 for f32).
All matmul operands are produced natively as f32r (walrus requires rounded
producers); PSUM accumulation stays fp32.
"""

import numpy as np
from contextlib import ExitStack

import concourse.bass as bass
import concourse.tile as tile
from concourse import bacc, mybir
from concourse.bass_utils import run_bass_kernel_spmd
from concourse.masks import make_upper_triangular

F32 = mybir.dt.float32
F32R = mybir.dt.float32r
AF = mybir.ActivationFunctionType

NCORES = 8
HD = 128
ROPE_THETA = 10000.0
USE_F32R = True


def rope_tables_T(Lsz):
    """cos/sin tables transposed to [HD, L], matching the fp32 reference."""
    half = np.arange(0, HD, 2).astype(np.float32) / np.float32(HD)
    inv_freq = (np.float32(1.0) / np.power(np.float32(ROPE_THETA), half,
                                           dtype=np.float32)).astype(np.float32)
    t = np.arange(Lsz, dtype=np.float32)
    freqs = np.outer(t, inv_freq).astype(np.float32)          # [L, HD/2]
    emb = np.concatenate([freqs, freqs], axis=1)              # [L, HD]
    cosT = np.ascontiguousarray(np.cos(emb).astype(np.float32).T)  # [HD, L]
    sinT = np.ascontiguousarray(np.sin(emb).astype(np.float32).T)
    return cosT, sinT


def rot_matrix_T():
    """R with rot(x) = R @ x = concat(-x2, x1); returns R.T (matmul lhsT)."""
    R = np.zeros((HD, HD), dtype=np.float32)
    h = HD // 2
    for d in range(h):
        R[d, d + h] = -1.0
        R[d + h, d] = 1.0
    return np.ascontiguousarray(R.T)


def build_attention_nc(Bsz, Lsz, Hsz, hpc, use_f32r=USE_F32R, repeat=1,
                       phases=(1, 1, 1)):
    """Build + compile the per-core Bass program (identical on all cores).

    repeat>1 re-emits the whole computation N times in one program — used
    only for timing (wall-time slope isolates device exec from dispatch
    overhead)."""
    f = Hsz // 128            # feature chunks of the model dim
    nt = Bsz * Lsz            # total tokens
    dloc = hpc * HD           # local head dims
    RC = 512                  # token chunk for projection + rope
    XC = 256                  # token chunk for x streaming (2 halves per RC)
    QT = 512                  # q tile for attention
    KCL = Lsz // 128          # k chunks per sequence
    scale = float(1.0 / np.sqrt(HD))
    MD = F32R if use_f32r else F32   # dtype for all matmul operands

    nc = bacc.Bacc("TRN2", target_bir_lowering=False, debug=False)

    xT = nc.dram_tensor("xT", [Hsz, nt], MD, kind="ExternalInput").ap()
    wqT = nc.dram_tensor("wqT", [Hsz, dloc], MD, kind="ExternalInput").ap()
    wkT = nc.dram_tensor("wkT", [Hsz, dloc], MD, kind="ExternalInput").ap()
    wvT = nc.dram_tensor("wvT", [Hsz, dloc], MD, kind="ExternalInput").ap()
    woT = nc.dram_tensor("woT", [dloc, Hsz], MD, kind="ExternalInput").ap()
    cosT = nc.dram_tensor("cosT", [HD, Lsz], F32, kind="ExternalInput").ap()
    sinT = nc.dram_tensor("sinT", [HD, Lsz], F32, kind="ExternalInput").ap()
    RT = nc.dram_tensor("RT", [HD, HD], MD, kind="ExternalInput").ap()
    y = nc.dram_tensor("y", [nt, Hsz], F32, kind="ExternalOutput").ap()

    with tile.TileContext(nc) as tc, \
         nc.allow_low_precision(reason="f32r matmul operands"), ExitStack() as ctx:
        wpool = ctx.enter_context(tc.tile_pool(name="wpool", bufs=1))
        cpool = ctx.enter_context(tc.tile_pool(name="cpool", bufs=1))
        xpool = ctx.enter_context(tc.tile_pool(name="xpool", bufs=3))
        spool = ctx.enter_context(tc.tile_pool(name="spool", bufs=1))
        work = ctx.enter_context(tc.tile_pool(name="work", bufs=2))
        psp = ctx.enter_context(tc.tile_pool(name="psp", bufs=2, space="PSUM"))

        # --- constants / weights resident in SBUF ---
        # DMA issue order matters: the first x tile goes first so the first
        # projection matmuls start ~6us in; weights stream per feature-chunk
        # behind it; w_o (needed only by the output projection) goes last.
        wq_s = wpool.tile([128, f, dloc], MD)
        wk_s = wpool.tile([128, f, dloc], MD)
        wv_s = wpool.tile([128, f, dloc], MD)
        wo_s = wpool.tile([128, hpc, Hsz], MD)
        xt0 = xpool.tile([128, f, XC], MD, tag="xt")
        nc.sync.dma_start(
            out=xt0, in_=xT.rearrange("(c p) n -> p c n", p=128)[:, :, 0:XC])
        for c in range(f):
            nc.sync.dma_start(
                out=wq_s[:, c, :],
                in_=wqT.rearrange("(c p) m -> p c m", p=128)[:, c, :])
        xt1 = xpool.tile([128, f, XC], MD, tag="xt")
        nc.sync.dma_start(
            out=xt1, in_=xT.rearrange("(c p) n -> p c n", p=128)[:, :, XC:2 * XC])
        for c in range(f):
            nc.sync.dma_start(
                out=wk_s[:, c, :],
                in_=wkT.rearrange("(c p) m -> p c m", p=128)[:, c, :])
        cos_s = cpool.tile([128, Lsz], F32)
        sin_s = cpool.tile([128, Lsz], F32)
        rt_s = cpool.tile([128, 128], MD)
        nc.sync.dma_start(out=sin_s, in_=sinT)
        nc.sync.dma_start(out=cos_s, in_=cosT)
        nc.sync.dma_start(out=rt_s, in_=RT)
        for c in range(f):
            nc.sync.dma_start(
                out=wv_s[:, c, :],
                in_=wvT.rearrange("(c p) m -> p c m", p=128)[:, c, :])
        nc.sync.dma_start(out=wo_s, in_=woT.rearrange("(h p) n -> p h n", p=128))
        _prefetched_xt = {0: [xt0, xt1]}
        tri_f = cpool.tile([128, 128], F32)
        make_upper_triangular(nc, tri_f, val=1.0, diag=True)
        ones_f = cpool.tile([128, 1], F32)
        nc.vector.memset(ones_f, 1.0)
        ones1_f = cpool.tile([1, 128], F32)
        nc.vector.memset(ones1_f, 1.0)
        if use_f32r:
            tri_s = cpool.tile([128, 128], MD)
            nc.vector.tensor_copy(tri_s, tri_f)
            ones_s = cpool.tile([128, 1], MD)
            nc.vector.tensor_copy(ones_s, ones_f)
            ones1_s = cpool.tile([1, 128], MD)
            nc.vector.tensor_copy(ones1_s, ones1_f)
        else:
            tri_s, ones_s, ones1_s = tri_f, ones_f, ones1_f

        for _rep in range(repeat):
         for b in range(Bsz):
            tb = b * Lsz  # token offset of this batch in xT / y

            # per-batch activation tensors (tags reused across b iterations)
            q_s = spool.tile([128, hpc, Lsz], MD, tag="q_s")
            k_s = spool.tile([128, hpc, Lsz], MD, tag="k_s")
            v_s = spool.tile([128, KCL, hpc, 128], MD, tag="v_s")

            # ---------------- P1: qkv projection + rope ----------------
            for rc in range(Lsz // RC if phases[0] else 0):
                t0 = rc * RC
                if _rep == 0 and b == 0 and rc in _prefetched_xt:
                    xts = _prefetched_xt.pop(rc)
                else:
                    xts = []
                    for half in range(RC // XC):
                        xt_t = xpool.tile([128, f, XC], MD, tag="xt")
                        nc.sync.dma_start(
                            out=xt_t,
                            in_=xT.rearrange("(c p) n -> p c n", p=128)[
                                :, :,
                                tb + t0 + half * XC: tb + t0 + (half + 1) * XC],
                        )
                        xts.append(xt_t)

                # q/k projections (head-dim-major) + rope, one (qk, h) at a time
                for dst, w_s in ((q_s, wq_s), (k_s, wk_s)):
                    for h in range(hpc):
                        p_ps = psp.tile([128, RC], F32, tag="proj", bufs=2)
                        first = True
                        for half in range(RC // XC):
                            cols = slice(half * XC, (half + 1) * XC)
                            for c in range(f):
                                nc.tensor.matmul(
                                    p_ps[:, cols],
                                    w_s[:, c, h * 128:(h + 1) * 128],
                                    xts[half][:, c, :],
                                    start=first,
                                    stop=(half == RC // XC - 1 and c == f - 1),
                                )
                                first = False
                        # rope: dst = p*cos + R @ (p*sin)
                        qs_t = work.tile([128, RC], MD, tag="ropesin", bufs=2)
                        nc.vector.tensor_mul(qs_t, p_ps, sin_s[:, t0:t0 + RC])
                        r_ps = psp.tile([128, RC], F32, tag="rot", bufs=2)
                        nc.tensor.matmul(r_ps, rt_s, qs_t, start=True, stop=True)
                        qc_t = work.tile([128, RC], F32, tag="ropecos", bufs=1)
                        nc.vector.tensor_mul(qc_t, p_ps, cos_s[:, t0:t0 + RC])
                        nc.vector.tensor_add(dst[:, h, t0:t0 + RC], qc_t, r_ps)

                # v projection (token-major, both heads at once)
                for half in range(RC // XC):
                    for m in range(XC // 128):
                        v_ps = psp.tile([128, dloc], F32, tag="vps", bufs=1)
                        for c in range(f):
                            nc.tensor.matmul(
                                v_ps,
                                xts[half][:, c, m * 128:(m + 1) * 128],
                                wv_s[:, c, :],
                                start=(c == 0), stop=(c == f - 1),
                            )
                        kc = (t0 + half * XC) // 128 + m
                        nc.scalar.activation(
                            v_s[:, kc, :, :].rearrange("p h d -> p (h d)"),
                            v_ps, AF.Copy)

            # ------- P2+P3: causal attention, then output projection -------
            # qt-outer / h-inner so each 512-token stripe's output projection
            # and y store overlap the next stripe's attention
            for qt in range(Lsz // QT if phases[1] else 0):
                o_st = spool.tile([128, hpc, QT], MD, tag="o_st", bufs=2)
                for h in range(hpc):
                    q0 = qt * QT
                    nkc = (q0 + QT) // 128
                    o_ps = psp.tile([128, QT], F32, tag="oacc", bufs=1)
                    d_ps = psp.tile([1, QT], F32, tag="vps", bufs=1)
                    for kc in range(nkc):
                        diag_j = kc - q0 // 128
                        c0 = max(0, diag_j * 128)
                        sc_ps = psp.tile([128, QT], F32, tag="sc", bufs=2)
                        nc.tensor.matmul(
                            sc_ps[:, c0:],
                            k_s[:, h, kc * 128:(kc + 1) * 128],
                            q_s[:, h, q0 + c0:q0 + QT],
                            start=True, stop=True,
                        )
                        e_t = work.tile([128, QT], MD, tag="exp", bufs=2)
                        nc.scalar.activation(e_t[:, c0:], sc_ps[:, c0:],
                                             AF.Exp, scale=scale)
                        if diag_j >= 0:
                            nc.vector.tensor_mul(e_t[:, c0:c0 + 128],
                                                 e_t[:, c0:c0 + 128], tri_s)
                        nc.tensor.matmul(
                            o_ps[:, c0:],
                            v_s[:, kc, h, :],
                            e_t[:, c0:],
                            start=(kc == 0), stop=(kc == nkc - 1),
                        )
                        nc.tensor.matmul(
                            d_ps[:, c0:],
                            ones_s,
                            e_t[:, c0:],
                            start=(kc == 0), stop=(kc == nkc - 1),
                        )
                    rc_t = work.tile([1, QT], MD, tag="recip", bufs=1)
                    nc.vector.reciprocal(rc_t, d_ps)
                    rb_ps = psp.tile([128, QT], F32, tag="proj", bufs=2)
                    nc.tensor.matmul(rb_ps, ones1_s, rc_t, start=True, stop=True)
                    rb_s = work.tile([128, QT], F32, tag="rb", bufs=1)
                    nc.scalar.activation(rb_s, rb_ps, AF.Copy)
                    nc.vector.tensor_mul(o_st[:, h, :], o_ps, rb_s)

                # output projection for this 512-token stripe (both heads)
                if phases[2]:
                    for tl in range(QT // 128):
                        t = qt * QT // 128 + tl
                        for n0 in range(0, Hsz, 512):
                            y_ps = psp.tile([128, 512], F32, tag="rot", bufs=2)
                            for h in range(hpc):
                                nc.tensor.matmul(
                                    y_ps,
                                    o_st[:, h, tl * 128:(tl + 1) * 128],
                                    wo_s[:, h, n0:n0 + 512],
                                    start=(h == 0), stop=(h == hpc - 1),
                                )
                            y_t = work.tile([128, 512], F32, tag="yst", bufs=3)
                            if (t + n0 // 512) % 2 == 0:
                                nc.scalar.activation(y_t, y_ps, AF.Copy)
                            else:
                                nc.vector.tensor_copy(y_t, y_ps)
                            nc.sync.dma_start(
                                out=y[tb + t * 128: tb + (t + 1) * 128,
                                      n0:n0 + 512],
                                in_=y_t)

    nc.compile()
    return nc


# ------------------------- host-side entry point -------------------------

_NC_CACHE = {}


def _get_nc(Bsz, Lsz, Hsz, hpc, use_f32r):
    key = (Bsz, Lsz, Hsz, hpc, use_f32r)
    if key not in _NC_CACHE:
        _NC_CACHE[key] = build_attention_nc(Bsz, Lsz, Hsz, hpc, use_f32r)
    return _NC_CACHE[key]


def make_in_maps(x, w_qkv, w_o, hpc):
    """Host-side sharding: per-core input dicts (all arrays np.float32)."""
    Bsz, Lsz, Hsz = x.shape
    dloc = hpc * HD
    xTh = np.ascontiguousarray(
        x.reshape(Bsz * Lsz, Hsz).T.astype(np.float32, copy=False))
    w_q, w_k, w_v = (w_qkv[i * Hsz:(i + 1) * Hsz] for i in range(3))
    cosTh, sinTh = rope_tables_T(Lsz)
    rth = rot_matrix_T()
    in_maps = []
    for c in range(NCORES):
        sl = slice(c * dloc, (c + 1) * dloc)
        in_maps.append({
            "xT": xTh,
            "wqT": np.ascontiguousarray(w_q[sl].T),
            "wkT": np.ascontiguousarray(w_k[sl].T),
            "wvT": np.ascontiguousarray(w_v[sl].T),
            "woT": np.ascontiguousarray(w_o[:, sl].T),
            "cosT": cosTh,
            "sinT": sinTh,
            "RT": rth,
        })
    return in_maps


def run(x, w_qkv, w_o, trace=False, use_f32r=USE_F32R):
    Bsz, Lsz, Hsz = x.shape
    hpc = (Hsz // HD) // NCORES
    nc = _get_nc(Bsz, Lsz, Hsz, hpc, use_f32r)
    in_maps = make_in_maps(np.asarray(x), np.asarray(w_qkv), np.asarray(w_o), hpc)
    res = run_bass_kernel_spmd(nc, in_maps, core_ids=list(range(NCORES)),
                               trace=trace)
    parts = np.stack([res.results[c]["y"] for c in range(NCORES)])
    y = parts.sum(axis=0, dtype=np.float64).astype(np.float32)
    return y.reshape(Bsz, Lsz, Hsz), res


def kernel(x, w_qkv, w_o):
    y, _ = run(x, w_qkv, w_o, trace=False)
    return y
